# revision 1
# baseline (speedup 1.0000x reference)
"""Trainium2 Bass kernel for nn_CLF_block (channel-attention block).

Reference computation (per batch item b, with x = concat([a,b], ch) in [256, N],
N = H*W = 16384):
    z  = w1 x + b1 1^T
    q  = w2 z + b2 1^T ;  k = w3 z + b3 1^T ;  v = w4 z + b4 1^T
    qk = q k^T ; attn = softmax(qk, -1) ; out = attn v

Algebraic restructuring (verified vs reference, max-rel ~1e-4):
    Gx = x x^T                [256,256]   (one pass over x)
    sx = x 1                  [256]
    u  = w1 sx ; s = u + N b1
    G  = w1 Gx w1^T + u b1^T + b1 u^T + N b1 b1^T        (= z z^T)
    qk = w2 G w3^T + (w2 s) b3^T + b2 (w3 s)^T + N b2 b3^T
    attn = softmax(qk)
    M  = attn w4 ; W = M w1 ; c0 = M b1 + attn b4
    out = W x + c0 1^T        (second pass over x)

So only two O(256*256*N) passes over x touch HBM-sized data; everything else is
256x256 algebra. HBM traffic per core = 16 MiB in + 16 MiB out (x stays in SBUF
between the passes) -> memory-bound.

Sharding: data-parallel over batch, one batch item per NeuronCore (B=8, 8 cores).
"""

import sys

if "/opt/trn_rl_repo" not in sys.path:
    sys.path.insert(0, "/opt/trn_rl_repo")

from contextlib import ExitStack

import numpy as np

import concourse.bass as bass
import concourse.mybir as mybir
import concourse.tile as tile
from concourse import bacc
from concourse.bass_utils import run_bass_kernel_spmd

F32 = mybir.dt.float32
F32R = mybir.dt.float32r
F16 = mybir.dt.float16
P = 128          # partitions / channel block
C = 256          # channels
NPIX = 128 * 128  # spatial positions per batch item
NPIECE = 16       # resident x pieces per input half
PIECE = NPIX // NPIECE   # 1024 cols per piece
NCHUNK = NPIX // P       # 128 gram chunks
OUTW = 2048       # output staging tile width
NT = 512          # matmul moving-operand width for pass 2


def _emit(nc, tc, ctx, d_in, d_out):
    """Emit the Tile program for one core (one batch item)."""
    wcat, ident = d_in["wcat"], d_in["ident"]
    xht_d, xlt_d, xr_d = d_in["xht"], d_in["xlt"], d_in["xr"]
    brows, bcols = d_in["brows"], d_in["bcols"]
    out_d = d_out["out"]

    const = ctx.enter_context(tc.tile_pool(name="const", bufs=1))
    xpool = ctx.enter_context(tc.tile_pool(name="xpool", bufs=1))

    # --- constants -------------------------------------------------------
    w_sb = []
    for k in range(2):
        wt = const.tile([P, 5 * C], F32, name=f"w_sb{k}", tag=f"w_sb{k}")
        nc.sync.dma_start(out=wt, in_=wcat[k * P:(k + 1) * P, :])
        w_sb.append(wt)
    w1t = [w_sb[k][:, 0 * C:1 * C] for k in range(2)]   # w1^T  [cin, o]
    w1r = [w_sb[k][:, 1 * C:2 * C] for k in range(2)]   # w1    [o, cin]
    w2t = [w_sb[k][:, 2 * C:3 * C] for k in range(2)]   # w2^T
    w3t = [w_sb[k][:, 3 * C:4 * C] for k in range(2)]   # w3^T
    w4r = [w_sb[k][:, 4 * C:5 * C] for k in range(2)]   # w4    [d', d]

    rows = []
    for r in range(5):
        rt = const.tile([1, C], F32, name=f"brow{r}", tag=f"brow{r}")
        nc.sync.dma_start(out=rt, in_=brows[r:r + 1, :])
        rows.append(rt)
    b1_row, nb1_row, b2_row, b3_row, nb3_row = rows

    bc_sb = []
    for k in range(2):
        bt = const.tile([P, 4], F32, name=f"bcol{k}", tag=f"bcol{k}")
        nc.sync.dma_start(out=bt, in_=bcols[k * P:(k + 1) * P, :])
        bc_sb.append(bt)
    b1_col = [bc_sb[k][:, 0:1] for k in range(2)]
    nb1_col = [bc_sb[k][:, 1:2] for k in range(2)]
    b4_col = [bc_sb[k][:, 2:3] for k in range(2)]

    ident_sb = const.tile([P, P], F32R, name="ident_sb", tag="ident_sb")
    nc.sync.dma_start(out=ident_sb, in_=ident[:, :])

    # --- resident f32r-rounded x for pass 2 (two channel halves) ---------
    xs = [[], []]
    for c in range(2):
        eng = nc.sync if c == 0 else nc.scalar
        for i in range(NPIECE):
            xt = xpool.tile([P, PIECE], F32R, name=f"x{c}_{i}", tag=f"x{c}_{i}")
            eng.dma_start(out=xt,
                          in_=xr_d[c * P:(c + 1) * P,
                                   i * PIECE:(i + 1) * PIECE])
            xs[c].append(xt)

    # --- pass 1: Gx = x x^T via host-side fp16 split + transpose ---------
    # Host supplies xht (= xh^T chunks, ones-augmented) and xlt (= xl^T
    # chunks). Gx = Xh Xh^T + C' + C'^T with C' = Xl Xh^T (error ~2^-22).
    # Column 256 of shh/c accumulates sxh/sxl (exact row sums).
    gx_sb = [
        const.tile([P, C + 1], F32, name=f"gx_sb{b}", tag=f"gx_sb{b}")
        for b in range(2)
    ]
    c_sb = [
        const.tile([P, C + 1], F32, name=f"c_sb{b}", tag=f"c_sb{b}")
        for b in range(2)
    ]
    CH_PP = PIECE // P  # gram chunks per piece
    with tc.tile_pool(name="gx_ps", bufs=1, space="PSUM") as gxp, \
         tc.tile_pool(name="xt_sb", bufs=3) as xtp:
        shh_ps = [
            gxp.tile([P, C + 1], F32, name=f"shh_ps{b}", tag=f"shh{b}")
            for b in range(2)
        ]
        c_ps = [
            gxp.tile([P, C + 1], F32, name=f"c_ps{b}", tag=f"cps{b}")
            for b in range(2)
        ]
        for i in range(NPIECE):
            xht_p = xtp.tile([P, CH_PP, C + 1], F16, name="xht_p", tag="xht_p")
            xlt_p = xtp.tile([P, CH_PP, C], F16, name="xlt_p", tag="xlt_p")
            nc.sync.dma_start(out=xht_p, in_=xht_d[i])
            nc.scalar.dma_start(out=xlt_p, in_=xlt_d[i])
            for g in range(CH_PP):
                ch = i * CH_PP + g
                for b in range(2):
                    bs = slice(b * P, (b + 1) * P)
                    nc.tensor.matmul(shh_ps[b], xht_p[:, g, bs],
                                     xht_p[:, g, :],
                                     start=(ch == 0),
                                     stop=(ch == NCHUNK - 1))
                    nc.tensor.matmul(c_ps[b], xlt_p[:, g, bs],
                                     xht_p[:, g, :],
                                     start=(ch == 0),
                                     stop=(ch == NCHUNK - 1))
        for b in range(2):
            nc.vector.tensor_copy(gx_sb[b], shh_ps[b])
            nc.vector.tensor_scalar_mul(c_sb[b], c_ps[b], 1.0 / 2048.0)

    # Gx += C' + C'^T ; col 256: sx = sxh + sxl
    with tc.tile_pool(name="fix_ps", bufs=2, space="PSUM") as fxp:
        for b in range(2):
            nc.vector.tensor_add(gx_sb[b], gx_sb[b], c_sb[b])
        for b in range(2):
            for jb in range(2):
                ctp = fxp.tile([P, P], F32, name="ctp", tag="ctp")
                nc.tensor.transpose(ctp, c_sb[jb][:, b * P:(b + 1) * P],
                                    ident_sb.bitcast(F32))  # C'^T block
                nc.vector.tensor_add(gx_sb[b][:, jb * P:(jb + 1) * P],
                                     gx_sb[b][:, jb * P:(jb + 1) * P], ctp)

    # Split the (large) diagonal out of Gx: products (Gx-D) w1 are ~100x
    # smaller, so the PE's per-product rounding no longer pollutes qk.
    # The diagonal term is applied exactly via per-partition multiplies.
    gxd = []
    for b in range(2):
        bs = slice(b * P, (b + 1) * P)
        dm = const.tile([P, P], F32, name=f"gxdm{b}", tag=f"gxdm{b}")
        nc.vector.tensor_mul(dm, gx_sb[b][:, bs], ident_sb.bitcast(F32))
        dcol = const.tile([P, 1], F32, name=f"gxd{b}", tag=f"gxd{b}")
        nc.vector.reduce_sum(out=dcol, in_=dm, axis=mybir.AxisListType.X)
        nc.vector.tensor_sub(gx_sb[b][:, bs], gx_sb[b][:, bs], dm)
        gxd.append(dcol)

    # --- tiny 256x256 algebra -------------------------------------------
    # All matrices in SBUF as two [128, *] row-blocks; vectors as [1, C] rows
    # or [128, 1] per-block columns.
    alg_sb = const  # persistent small tiles live in the const pool

    with tc.tile_pool(name="alg_ps", bufs=3, space="PSUM") as ap:
        # u_row = (w1 sx)^T : lhsT = sx col (gx col 256), rhs = w1t
        u_row = alg_sb.tile([1, C], F32, name="u_row", tag="u_row")
        u_ps = ap.tile([1, C], F32, name="u_ps", tag="alg")
        for k in range(2):
            nc.tensor.matmul(u_ps, gx_sb[k][:, C:C + 1],
                             w1t[k].bitcast(F32),
                             start=(k == 0), stop=(k == 1))
        nc.vector.tensor_copy(u_row, u_ps)

        # U = (w1 Gx)^T : U[c, o] ; lhsT = Gx[c' blk k, c blk b], rhs = w1t[k]
        u_sb = []
        for b in range(2):
            ups = ap.tile([P, C], F32, name="ups", tag="alg")
            for k in range(2):
                nc.tensor.matmul(ups, gx_sb[k][:, b * P:(b + 1) * P],
                                 w1t[k].bitcast(F32),
                                 start=(k == 0), stop=(k == 1))
            ud = alg_sb.tile([P, C], F32, name=f"u_d{b}", tag=f"u_d{b}")
            nc.vector.tensor_scalar_mul(ud, w1t[b], gxd[b])
            ut = alg_sb.tile([P, C], F32, name=f"u_sb{b}", tag=f"u_sb{b}")
            nc.vector.tensor_add(ut, ups, ud)
            u_sb.append(ut)

        # G = U^T w1^T (+ rank-1 bias terms); u as column in separate psum
        g_sb = []
        g_diag = []
        for b in range(2):
            gps = ap.tile([P, C], F32, name="gps", tag="alg")
            ucps = ap.tile([P, 1], F32, name="ucps", tag="algsmall", bufs=2)
            for k in range(2):
                nc.tensor.matmul(gps,
                                 u_sb[k][:, b * P:(b + 1) * P].bitcast(F32),
                                 w1t[k].bitcast(F32), start=(k == 0),
                                 stop=False)
                # u_col block b: lhsT = w1t[k][:, b-slice], rhs = sx col
                nc.tensor.matmul(ucps,
                                 w1t[k][:, b * P:(b + 1) * P].bitcast(F32),
                                 gx_sb[k][:, C:C + 1],
                                 start=(k == 0), stop=(k == 1))
            nc.tensor.matmul(gps, u_row[:, b * P:(b + 1) * P], b1_row,
                             start=False, stop=False)
            nc.tensor.matmul(gps, b1_row[:, b * P:(b + 1) * P], u_row,
                             start=False, stop=False)
            nc.tensor.matmul(gps, b1_row[:, b * P:(b + 1) * P],
                             nb1_row, start=False, stop=True)
            gt = alg_sb.tile([P, C + 1], F32, name=f"g_sb{b}", tag=f"g_sb{b}")
            nc.vector.tensor_copy(gt[:, 0:C], gps)
            nc.vector.tensor_copy(gt[:, C:C + 1], ucps)
            bs = slice(b * P, (b + 1) * P)
            gdm = alg_sb.tile([P, P], F32, name=f"gdm{b}", tag=f"gdm{b}")
            nc.vector.tensor_mul(gdm, gt[:, bs], ident_sb.bitcast(F32))
            gdc = alg_sb.tile([P, 1], F32, name=f"gd{b}", tag=f"gd{b}")
            nc.vector.reduce_sum(out=gdc, in_=gdm, axis=mybir.AxisListType.X)
            nc.vector.tensor_sub(gt[:, bs], gt[:, bs], gdm)
            g_sb.append(gt)
            g_diag.append(gdc)

        # s_col = u_col + N*b1 (per block)
        s_col = []
        for k in range(2):
            st = alg_sb.tile([P, 1], F32, name=f"s_col{k}", tag=f"s_col{k}")
            nc.vector.tensor_add(st, g_sb[k][:, C:C + 1], nb1_col[k].bitcast(F32))
            s_col.append(st)

        # w2s_row = (w2 s)^T, w3s_row = (w3 s)^T
        w2s_row = alg_sb.tile([1, C], F32, name="w2s_row", tag="w2s_row")
        w3s_row = alg_sb.tile([1, C], F32, name="w3s_row", tag="w3s_row")
        for dst, wt in ((w2s_row, w2t), (w3s_row, w3t)):
            vps = ap.tile([1, C], F32, name="vps", tag="alg")
            for k in range(2):
                nc.tensor.matmul(vps, s_col[k].bitcast(F32),
                                 wt[k].bitcast(F32),
                                 start=(k == 0), stop=(k == 1))
            nc.vector.tensor_copy(dst, vps)

        # U2 = (w2 G)^T
        u2_sb = []
        for b in range(2):
            u2ps = ap.tile([P, C], F32, name="u2ps", tag="alg")
            for k in range(2):
                nc.tensor.matmul(u2ps, g_sb[k][:, b * P:(b + 1) * P].bitcast(F32),
                                 w2t[k].bitcast(F32),
                                 start=(k == 0), stop=(k == 1))
            u2d = alg_sb.tile([P, C], F32, name=f"u2_d{b}", tag=f"u2_d{b}")
            nc.vector.tensor_scalar_mul(u2d, w2t[b], g_diag[b])
            u2t = alg_sb.tile([P, C], F32, name=f"u2_sb{b}", tag=f"u2_sb{b}")
            nc.vector.tensor_add(u2t, u2ps, u2d)
            u2_sb.append(u2t)

        # qk = U2^T w3^T + rank-1 terms ; then softmax rows
        attn_sb = []
        for b in range(2):
            qkps = ap.tile([P, C], F32, name="qkps", tag="alg")
            for k in range(2):
                nc.tensor.matmul(qkps,
                                 u2_sb[k][:, b * P:(b + 1) * P].bitcast(F32),
                                 w3t[k].bitcast(F32), start=(k == 0),
                                 stop=False)
            nc.tensor.matmul(qkps, w2s_row[:, b * P:(b + 1) * P], b3_row,
                             start=False, stop=False)
            nc.tensor.matmul(qkps, b2_row[:, b * P:(b + 1) * P], w3s_row,
                             start=False, stop=False)
            nc.tensor.matmul(qkps, b2_row[:, b * P:(b + 1) * P], nb3_row,
                             start=False, stop=True)

            negmax = alg_sb.tile([P, 1], F32, name=f"negmax{b}", tag=f"nm{b}")
            nc.vector.tensor_reduce(
                out=negmax, in_=qkps, op=mybir.AluOpType.max,
                axis=mybir.AxisListType.X, negate=True,
            )
            expq = alg_sb.tile([P, C], F32, name=f"expq{b}", tag=f"expq{b}")
            nc.scalar.activation(
                out=expq, in_=qkps, func=mybir.ActivationFunctionType.Exp,
                bias=negmax, scale=1.0,
            )
            denom = alg_sb.tile([P, 1], F32, name=f"denom{b}", tag=f"dn{b}")
            nc.vector.reduce_sum(out=denom, in_=expq,
                                 axis=mybir.AxisListType.X)
            rden = alg_sb.tile([P, 1], F32, name=f"rden{b}", tag=f"rd{b}")
            nc.vector.reciprocal(rden, denom)
            at = alg_sb.tile([P, C], F32, name=f"attn{b}", tag=f"attn{b}")
            nc.vector.tensor_scalar_mul(at, expq, rden)
            attn_sb.append(at)

        # attn^T (4 PE transposes)
        attnT_sb = [
            alg_sb.tile([P, C], F32, name=f"attnT{j}", tag=f"attnT{j}")
            for j in range(2)
        ]
        for b in range(2):
            for j in range(2):
                tps = ap.tile([P, P], F32, name="tps", tag="algtp", bufs=2)
                nc.tensor.transpose(tps,
                                    attn_sb[b][:, j * P:(j + 1) * P],
                                    ident_sb.bitcast(F32))
                nc.vector.tensor_copy(attnT_sb[j][:, b * P:(b + 1) * P], tps)

        # M^T = w4-as-lhsT @ attn^T
        mt_sb = []
        for b in range(2):
            mps = ap.tile([P, C], F32, name="mps", tag="alg")
            for k in range(2):
                nc.tensor.matmul(mps, w4r[k][:, b * P:(b + 1) * P],
                                 (attnT_sb[k]), start=(k == 0), stop=(k == 1))
            mt = alg_sb.tile([P, C], F32, name=f"mt_sb{b}", tag=f"mt_sb{b}")
            nc.vector.tensor_copy(mt, mps)
            mt_sb.append(mt)

        # W^T = w1-as-lhsT @ M^T
        wt_sb = []
        for b in range(2):
            wps = ap.tile([P, C], F32, name="wps", tag="alg")
            for k in range(2):
                nc.tensor.matmul(wps, w1r[k][:, b * P:(b + 1) * P], mt_sb[k],
                                 start=(k == 0), stop=(k == 1))
            wt_ = alg_sb.tile([P, C], F32R, name=f"wt_sb{b}", tag=f"wt_sb{b}")
            nc.vector.tensor_copy(wt_, wps)
            wt_sb.append(wt_)

        # c0_col = M b1 + attn b4 (per block)
        c0_col = []
        for b in range(2):
            cps = ap.tile([P, 1], F32, name="cps", tag="alg")
            for k in range(2):
                nc.tensor.matmul(cps, mt_sb[k][:, b * P:(b + 1) * P].bitcast(F32),
                                 b1_col[k].bitcast(F32), start=(k == 0),
                                 stop=False)
            for k in range(2):
                nc.tensor.matmul(cps,
                                 attnT_sb[k][:, b * P:(b + 1) * P].bitcast(F32),
                                 b4_col[k].bitcast(F32), start=False,
                                 stop=(k == 1))
            ct = alg_sb.tile([P, 1], F32, name=f"c0_col{b}", tag=f"c0_col{b}")
            nc.vector.tensor_copy(ct, cps)
            c0_col.append(ct)

    # --- pass 2: out = W x + c0 1^T -------------------------------------
    # rhs x slices are rounded to f32r on the fly (7.6e-6 perturbation).
    with tc.tile_pool(name="o_ps", bufs=4, space="PSUM") as ops, \
         tc.tile_pool(name="o_sb", bufs=3) as osb, \
         tc.tile_pool(name="xr_sb", bufs=2) as xrp:
        nsub = PIECE // NT  # psum tiles per staging tile
        for i in range(NPIECE):
            xr = []
            for k in range(2):
                xrt = xrp.tile([P, PIECE], F32R, name=f"xr{k}", tag=f"xr{k}")
                nc.vector.tensor_copy(xrt, xs[k][i])
                xr.append(xrt)
            for b in range(2):
                ot = osb.tile([P, PIECE], F32, name="ot", tag="ot")
                pst = [
                    ops.tile([P, NT], F32, name="pst", tag="pst")
                    for _ in range(nsub)
                ]
                for k in range(2):
                    for t in range(nsub):
                        nc.tensor.matmul(
                            pst[t],
                            wt_sb[k][:, b * P:(b + 1) * P],
                            xr[k][:, t * NT:(t + 1) * NT],
                            start=(k == 0),
                            stop=(k == 1),
                        )
                for t in range(nsub):
                    nc.scalar.activation(
                        out=ot[:, t * NT:(t + 1) * NT], in_=pst[t],
                        func=mybir.ActivationFunctionType.Identity,
                        bias=c0_col[b], scale=1.0,
                    )
                (nc.sync if b == 0 else nc.scalar).dma_start(
                    out=out_d[b * P:(b + 1) * P, i * PIECE:(i + 1) * PIECE],
                    in_=ot,
                )


def build_program(enable_asserts=False):
    nc = bacc.Bacc(
        "TRN2",
        target_bir_lowering=False,
        debug=False,
        enable_asserts=enable_asserts,
        num_devices=8,
    )
    d_in = {
        "xht": nc.dram_tensor("xht", [NPIECE, P, PIECE // P, C + 1],
                              mybir.dt.float16, kind="ExternalInput").ap(),
        "xlt": nc.dram_tensor("xlt", [NPIECE, P, PIECE // P, C],
                              mybir.dt.float16, kind="ExternalInput").ap(),
        "xr": nc.dram_tensor("xr", [C, NPIX], F32R,
                             kind="ExternalInput").ap(),
        "wcat": nc.dram_tensor("wcat", [C, 5 * C], F32,
                               kind="ExternalInput").ap(),
        "brows": nc.dram_tensor("brows", [5, C], F32,
                                kind="ExternalInput").ap(),
        "bcols": nc.dram_tensor("bcols", [C, 4], F32,
                                kind="ExternalInput").ap(),
        "ident": nc.dram_tensor("ident", [P, P], F32R,
                                kind="ExternalInput").ap(),
    }
    d_out = {
        "out": nc.dram_tensor("out", [C, NPIX], F32,
                              kind="ExternalOutput").ap(),

    }
    with tile.TileContext(nc) as tc, ExitStack() as ctx:
        _emit(nc, tc, ctx, d_in, d_out)
    nc.compile()
    return nc


def _round_f32r(x):
    """Round fp32 to the FP32R-representable set (hi-bf16 + lo-bf16)."""
    import ml_dtypes

    x = np.asarray(x, np.float32)
    hi = x.astype(ml_dtypes.bfloat16).astype(np.float32)
    lo = (x - hi).astype(ml_dtypes.bfloat16).astype(np.float32)
    return hi + lo


def make_in_maps(a, b, w1, b1, w2, b2, w3, b3, w4, b4):
    N = NPIX
    f = np.float32
    wcat = np.concatenate([w1.T, w1, w2.T, w3.T, w4],
                          axis=1).astype(f, copy=False)
    brows = np.stack([b1, N * b1, b2, b3, N * b3]).astype(f, copy=False)
    bcols = np.stack([b1, N * b1, b4, np.ones(C, f)], axis=1).astype(f)
    ident = np.eye(P, dtype=f)
    B = a.shape[0]
    g = PIECE // P
    in_maps = []
    for i in range(B):
        x = np.concatenate([a[i].reshape(P, N), b[i].reshape(P, N)], axis=0)
        xh = x.astype(np.float16)
        # scale xl into fp16 normal range (PE flushes fp16 subnormals);
        # the kernel rescales the C' term by 1/2048.
        xl = ((x - xh.astype(f)) * 2048.0).astype(np.float16)
        xht = np.ascontiguousarray(
            xh.T.reshape(NPIECE, g, P, C).transpose(0, 2, 1, 3))
        ones = np.ones((NPIECE, P, g, 1), np.float16)
        xht = np.ascontiguousarray(np.concatenate([xht, ones], axis=3))
        xlt = np.ascontiguousarray(
            xl.T.reshape(NPIECE, g, P, C).transpose(0, 2, 1, 3))
        in_maps.append({
            "xht": xht,
            "xlt": xlt,
            "xr": _round_f32r(x),
            "wcat": wcat,
            "brows": brows,
            "bcols": bcols,
            "ident": ident,
        })
    return in_maps


_CACHE = {}


def kernel(a, b, w1, b1, w2, b2, w3, b3, w4, b4, _trace=False):
    a = np.asarray(a, dtype=np.float32)
    b = np.asarray(b, dtype=np.float32)
    args = [np.asarray(t, dtype=np.float32)
            for t in (w1, b1, w2, b2, w3, b3, w4, b4)]
    if "nc" not in _CACHE:
        _CACHE["nc"] = build_program()
    nc = _CACHE["nc"]
    in_maps = make_in_maps(a, b, *args)
    res = run_bass_kernel_spmd(nc, in_maps, core_ids=list(range(8)),
                               trace=_trace)
    B, Ch, H, W = a.shape
    out = np.stack([r["out"].reshape(C, H, W) for r in res.results])
    if _trace:
        _CACHE["last_results"] = res
    return out



# revision 8
# speedup vs baseline: 1.5675x; 1.5675x over previous
"""Trainium2 Bass kernel for nn_CLF_block (channel-attention block).

Reference computation (per batch item b, with x = concat([a,b], ch) in [256, N],
N = H*W = 16384):
    z  = w1 x + b1 1^T
    q  = w2 z + b2 1^T ;  k = w3 z + b3 1^T ;  v = w4 z + b4 1^T
    qk = q k^T ; attn = softmax(qk, -1) ; out = attn v

Algebraic restructuring (verified vs reference):
    Gx = x x^T                [256,256]   (one pass over x)
    sx = x 1                  [256]
    u  = w1 sx ; s = u + N b1
    G  = w1 Gx w1^T + u b1^T + b1 u^T + N b1 b1^T        (= z z^T)
    qk = w2 G w3^T + (w2 s) b3^T + b2 (w3 s)^T + N b2 b3^T
    attn = softmax(qk)
    M  = attn w4 ; W = M w1 ; c0 = M b1 + attn b4
    out = W x + c0 1^T        (second pass over x)

V2 data plan (per core; tolerance is 2e-2 so fp16 x suffices end-to-end,
numpy-verified rel err ~2.9e-3):
  - x is loaded ONCE from HBM, as transposed fp16 pieces xht [n, c] with a
    ones column (8.4 MiB).  The Gram accumulates directly from these.
  - The natural-layout fp16 copy of x needed by pass 2 is built on-chip with
    PE transposes of the same streamed chunks (fp16 transpose = 1 cyc/row).
  - Output is written as fp16 (8.4 MiB) and upcast on host.
  Total HBM traffic ~18 MiB/core vs 51.7 MiB for V1.

Sharding: data-parallel over batch, one batch item per NeuronCore (B=8, 8 cores).
"""

import sys

if "/opt/trn_rl_repo" not in sys.path:
    sys.path.insert(0, "/opt/trn_rl_repo")

from contextlib import ExitStack

import numpy as np

import concourse.bass as bass
import concourse.mybir as mybir
import concourse.tile as tile
from concourse import bacc
from concourse.bass_utils import run_bass_kernel_spmd

F32 = mybir.dt.float32
F16 = mybir.dt.float16
P = 128          # partitions / channel block
C = 256          # channels
NPIX = 128 * 128  # spatial positions per batch item
NPIECE = 16       # streamed x^T pieces
PIECE = NPIX // NPIECE   # 1024 cols per piece
NCHUNK = NPIX // P       # 128 gram chunks
CH_PP = PIECE // P       # gram chunks per piece (8)
NT = 512          # matmul moving-operand width for pass 2


def _emit(nc, tc, ctx, d_in, d_out):
    """Emit the Tile program for one core (one batch item)."""
    wcat, ident, ident16 = d_in["wcat"], d_in["ident"], d_in["ident16"]
    xht_d = d_in["xht"]
    brows, bcols = d_in["brows"], d_in["bcols"]
    out_d = d_out["out"]

    const = ctx.enter_context(tc.tile_pool(name="const", bufs=1))
    xpool = ctx.enter_context(tc.tile_pool(name="xpool", bufs=1))

    # DMA queues to rotate over for bulk transfers
    qs = [nc.sync, nc.scalar]

    # --- constants -------------------------------------------------------
    w_sb = []
    for k in range(2):
        wt = const.tile([P, 5 * C], F32, name=f"w_sb{k}", tag=f"w_sb{k}")
        nc.sync.dma_start(out=wt, in_=wcat[k * P:(k + 1) * P, :])
        w_sb.append(wt)
    w1t = [w_sb[k][:, 0 * C:1 * C] for k in range(2)]   # w1^T  [cin, o]
    w1r = [w_sb[k][:, 1 * C:2 * C] for k in range(2)]   # w1    [o, cin]
    w2t = [w_sb[k][:, 2 * C:3 * C] for k in range(2)]   # w2^T
    w3t = [w_sb[k][:, 3 * C:4 * C] for k in range(2)]   # w3^T
    w4r = [w_sb[k][:, 4 * C:5 * C] for k in range(2)]   # w4    [d', d]

    rows = []
    for r in range(5):
        rt = const.tile([1, C], F32, name=f"brow{r}", tag=f"brow{r}")
        nc.sync.dma_start(out=rt, in_=brows[r:r + 1, :])
        rows.append(rt)
    b1_row, nb1_row, b2_row, b3_row, nb3_row = rows

    bc_sb = []
    for k in range(2):
        bt = const.tile([P, 4], F32, name=f"bcol{k}", tag=f"bcol{k}")
        nc.sync.dma_start(out=bt, in_=bcols[k * P:(k + 1) * P, :])
        bc_sb.append(bt)
    b1_col = [bc_sb[k][:, 0:1] for k in range(2)]
    nb1_col = [bc_sb[k][:, 1:2] for k in range(2)]
    b4_col = [bc_sb[k][:, 2:3] for k in range(2)]

    ident_sb = const.tile([P, P], F32, name="ident_sb", tag="ident_sb")
    nc.sync.dma_start(out=ident_sb, in_=ident[:, :])
    ident16_sb = const.tile([P, P], F16, name="ident16_sb", tag="ident16_sb")
    nc.sync.dma_start(out=ident16_sb, in_=ident16[:, :])

    # natural-layout fp16 x, reconstructed on-chip (pass-2 moving operand)
    xs = [[None] * NPIECE, [None] * NPIECE]

    # --- pass 1: stream x^T pieces; Gram + on-chip transposes ------------
    # Gram is symmetric: accumulate block-row 0 full ([128, 257], col 256 =
    # sx via the ones column) and only the (1,1)+sx part of block-row 1
    # ([128, 129]); block (1,0) is recovered by one PE transpose at the end.
    gx_sb = [
        const.tile([P, C + 1], F32, name=f"gx_sb{b}", tag=f"gx_sb{b}")
        for b in range(2)
    ]
    with tc.tile_pool(name="gx_ps", bufs=1, space="PSUM") as gxp, \
         tc.tile_pool(name="nat_ps", bufs=2, space="PSUM") as natp, \
         tc.tile_pool(name="xt_sb", bufs=3) as xtp:
        shh0 = gxp.tile([P, C + 1], F32, name="shh0", tag="shh0")
        shh1 = gxp.tile([P, P + 1], F32, name="shh1", tag="shh1")
        for i in range(NPIECE):
            xht_p = xtp.tile([P, CH_PP, C + 1], F16, name="xht_p", tag="xht_p")
            qs[i % 2].dma_start(out=xht_p, in_=xht_d[i])
            nat_ps = [
                natp.tile([P, PIECE], F16, name=f"nat{c}", tag=f"nat{c}")
                for c in range(2)
            ]
            for g in range(CH_PP):
                ch = i * CH_PP + g
                chunk = xht_p[:, g, :]
                nc.tensor.matmul(shh0, chunk[:, 0:P], chunk,
                                 start=(ch == 0), stop=(ch == NCHUNK - 1))
                nc.tensor.matmul(shh1, chunk[:, P:C], chunk[:, P:C + 1],
                                 start=(ch == 0), stop=(ch == NCHUNK - 1))
                for c in range(2):
                    nc.tensor.transpose(nat_ps[c][:, g * P:(g + 1) * P],
                                        chunk[:, c * P:(c + 1) * P],
                                        ident16_sb)
            for c in range(2):
                xt = xpool.tile([P, PIECE], F16, name=f"x{c}_{i}",
                                tag=f"x{c}_{i}")
                eng = (nc.vector, nc.scalar)[(2 * i + c) % 2]
                if eng is nc.scalar:
                    eng.copy(xt, nat_ps[c])
                else:
                    eng.tensor_copy(xt, nat_ps[c])
                xs[c][i] = xt

        # unload Gram accumulators; rebuild block (1,0) by transpose
        with tc.tile_pool(name="gfix_ps", bufs=1, space="PSUM") as gfp:
            nc.vector.tensor_copy(gx_sb[0], shh0)
            nc.scalar.copy(gx_sb[1][:, P:C + 1], shh1)
            g10 = gfp.tile([P, P], F32, name="g10", tag="g10")
            nc.tensor.transpose(g10, gx_sb[0][:, P:C], ident_sb)
            nc.vector.tensor_copy(gx_sb[1][:, 0:P], g10)

    # Split the (large) diagonal out of Gx: products (Gx-D) w1 are ~100x
    # smaller, so the PE's per-product rounding no longer pollutes qk.
    # The diagonal term is applied exactly via per-partition multiplies.
    gxd = []
    for b in range(2):
        bs = slice(b * P, (b + 1) * P)
        dm = const.tile([P, P], F32, name=f"gxdm{b}", tag=f"gxdm{b}")
        nc.vector.tensor_mul(dm, gx_sb[b][:, bs], ident_sb)
        dcol = const.tile([P, 1], F32, name=f"gxd{b}", tag=f"gxd{b}")
        nc.vector.reduce_sum(out=dcol, in_=dm, axis=mybir.AxisListType.X)
        nc.vector.tensor_sub(gx_sb[b][:, bs], gx_sb[b][:, bs], dm)
        gxd.append(dcol)

    # --- tiny 256x256 algebra -------------------------------------------
    # All matrices in SBUF as two [128, *] row-blocks; vectors as [1, C] rows
    # or [128, 1] per-block columns.
    alg_sb = const  # persistent small tiles live in the const pool

    with tc.tile_pool(name="alg_ps", bufs=3, space="PSUM") as ap:
        # u_row = (w1 sx)^T : lhsT = sx col (gx col 256), rhs = w1t
        u_row = alg_sb.tile([1, C], F32, name="u_row", tag="u_row")
        u_ps = ap.tile([1, C], F32, name="u_ps", tag="alg")
        for k in range(2):
            nc.tensor.matmul(u_ps, gx_sb[k][:, C:C + 1], w1t[k],
                             start=(k == 0), stop=(k == 1))
        nc.vector.tensor_copy(u_row, u_ps)

        # U = (w1 Gx)^T : U[c, o] ; lhsT = Gx[c' blk k, c blk b], rhs = w1t[k]
        u_sb = []
        for b in range(2):
            ups = ap.tile([P, C], F32, name="ups", tag="alg")
            for k in range(2):
                nc.tensor.matmul(ups, gx_sb[k][:, b * P:(b + 1) * P], w1t[k],
                                 start=(k == 0), stop=(k == 1))
            ud = alg_sb.tile([P, C], F32, name=f"u_d{b}", tag=f"u_d{b}")
            nc.vector.tensor_scalar_mul(ud, w1t[b], gxd[b])
            ut = alg_sb.tile([P, C], F32, name=f"u_sb{b}", tag=f"u_sb{b}")
            nc.vector.tensor_add(ut, ups, ud)
            u_sb.append(ut)

        # G = U^T w1^T (+ rank-1 bias terms); u as column in separate psum
        g_sb = []
        g_diag = []
        for b in range(2):
            gps = ap.tile([P, C], F32, name="gps", tag="alg")
            ucps = ap.tile([P, 1], F32, name="ucps", tag="algsmall", bufs=2)
            for k in range(2):
                nc.tensor.matmul(gps, u_sb[k][:, b * P:(b + 1) * P], w1t[k],
                                 start=(k == 0), stop=False)
                # u_col block b: lhsT = w1t[k][:, b-slice], rhs = sx col
                nc.tensor.matmul(ucps, w1t[k][:, b * P:(b + 1) * P],
                                 gx_sb[k][:, C:C + 1],
                                 start=(k == 0), stop=(k == 1))
            nc.tensor.matmul(gps, u_row[:, b * P:(b + 1) * P], b1_row,
                             start=False, stop=False)
            nc.tensor.matmul(gps, b1_row[:, b * P:(b + 1) * P], u_row,
                             start=False, stop=False)
            nc.tensor.matmul(gps, b1_row[:, b * P:(b + 1) * P],
                             nb1_row, start=False, stop=True)
            gt = alg_sb.tile([P, C + 1], F32, name=f"g_sb{b}", tag=f"g_sb{b}")
            nc.vector.tensor_copy(gt[:, 0:C], gps)
            nc.vector.tensor_copy(gt[:, C:C + 1], ucps)
            bs = slice(b * P, (b + 1) * P)
            gdm = alg_sb.tile([P, P], F32, name=f"gdm{b}", tag=f"gdm{b}")
            nc.vector.tensor_mul(gdm, gt[:, bs], ident_sb)
            gdc = alg_sb.tile([P, 1], F32, name=f"gd{b}", tag=f"gd{b}")
            nc.vector.reduce_sum(out=gdc, in_=gdm, axis=mybir.AxisListType.X)
            nc.vector.tensor_sub(gt[:, bs], gt[:, bs], gdm)
            g_sb.append(gt)
            g_diag.append(gdc)

        # s_col = u_col + N*b1 (per block)
        s_col = []
        for k in range(2):
            st = alg_sb.tile([P, 1], F32, name=f"s_col{k}", tag=f"s_col{k}")
            nc.vector.tensor_add(st, g_sb[k][:, C:C + 1], nb1_col[k])
            s_col.append(st)

        # w2s_row = (w2 s)^T, w3s_row = (w3 s)^T
        w2s_row = alg_sb.tile([1, C], F32, name="w2s_row", tag="w2s_row")
        w3s_row = alg_sb.tile([1, C], F32, name="w3s_row", tag="w3s_row")
        for dst, wt in ((w2s_row, w2t), (w3s_row, w3t)):
            vps = ap.tile([1, C], F32, name="vps", tag="alg")
            for k in range(2):
                nc.tensor.matmul(vps, s_col[k], wt[k],
                                 start=(k == 0), stop=(k == 1))
            nc.vector.tensor_copy(dst, vps)

        # U2 = (w2 G)^T
        u2_sb = []
        for b in range(2):
            u2ps = ap.tile([P, C], F32, name="u2ps", tag="alg")
            for k in range(2):
                nc.tensor.matmul(u2ps, g_sb[k][:, b * P:(b + 1) * P], w2t[k],
                                 start=(k == 0), stop=(k == 1))
            u2d = alg_sb.tile([P, C], F32, name=f"u2_d{b}", tag=f"u2_d{b}")
            nc.vector.tensor_scalar_mul(u2d, w2t[b], g_diag[b])
            u2t = alg_sb.tile([P, C], F32, name=f"u2_sb{b}", tag=f"u2_sb{b}")
            nc.vector.tensor_add(u2t, u2ps, u2d)
            u2_sb.append(u2t)

        # qk = U2^T w3^T + rank-1 terms ; then softmax rows
        attn_sb = []
        for b in range(2):
            qkps = ap.tile([P, C], F32, name="qkps", tag="alg")
            for k in range(2):
                nc.tensor.matmul(qkps, u2_sb[k][:, b * P:(b + 1) * P], w3t[k],
                                 start=(k == 0), stop=False)
            nc.tensor.matmul(qkps, w2s_row[:, b * P:(b + 1) * P], b3_row,
                             start=False, stop=False)
            nc.tensor.matmul(qkps, b2_row[:, b * P:(b + 1) * P], w3s_row,
                             start=False, stop=False)
            nc.tensor.matmul(qkps, b2_row[:, b * P:(b + 1) * P],
                             nb3_row, start=False, stop=True)

            negmax = alg_sb.tile([P, 1], F32, name=f"negmax{b}", tag=f"nm{b}")
            nc.vector.tensor_reduce(
                out=negmax, in_=qkps, op=mybir.AluOpType.max,
                axis=mybir.AxisListType.X, negate=True,
            )
            expq = alg_sb.tile([P, C], F32, name=f"expq{b}", tag=f"expq{b}")
            nc.scalar.activation(
                out=expq, in_=qkps, func=mybir.ActivationFunctionType.Exp,
                bias=negmax, scale=1.0,
            )
            denom = alg_sb.tile([P, 1], F32, name=f"denom{b}", tag=f"dn{b}")
            nc.vector.reduce_sum(out=denom, in_=expq,
                                 axis=mybir.AxisListType.X)
            rden = alg_sb.tile([P, 1], F32, name=f"rden{b}", tag=f"rd{b}")
            nc.vector.reciprocal(rden, denom)
            at = alg_sb.tile([P, C], F32, name=f"attn{b}", tag=f"attn{b}")
            nc.vector.tensor_scalar_mul(at, expq, rden)
            attn_sb.append(at)

        # attn^T (4 PE transposes)
        attnT_sb = [
            alg_sb.tile([P, C], F32, name=f"attnT{j}", tag=f"attnT{j}")
            for j in range(2)
        ]
        for b in range(2):
            for j in range(2):
                tps = ap.tile([P, P], F32, name="tps", tag="algtp", bufs=2)
                nc.tensor.transpose(tps, attn_sb[b][:, j * P:(j + 1) * P],
                                    ident_sb)
                nc.vector.tensor_copy(attnT_sb[j][:, b * P:(b + 1) * P], tps)

        # M^T = w4-as-lhsT @ attn^T
        mt_sb = []
        for b in range(2):
            mps = ap.tile([P, C], F32, name="mps", tag="alg")
            for k in range(2):
                nc.tensor.matmul(mps, w4r[k][:, b * P:(b + 1) * P],
                                 attnT_sb[k], start=(k == 0), stop=(k == 1))
            mt = alg_sb.tile([P, C], F32, name=f"mt_sb{b}", tag=f"mt_sb{b}")
            nc.vector.tensor_copy(mt, mps)
            mt_sb.append(mt)

        # W^T = w1-as-lhsT @ M^T  (stored fp16 for the pass-2 matmuls)
        wt_sb = []
        for b in range(2):
            wps = ap.tile([P, C], F32, name="wps", tag="alg")
            for k in range(2):
                nc.tensor.matmul(wps, w1r[k][:, b * P:(b + 1) * P], mt_sb[k],
                                 start=(k == 0), stop=(k == 1))
            wt_ = alg_sb.tile([P, C], F16, name=f"wt_sb{b}", tag=f"wt_sb{b}")
            nc.vector.tensor_copy(wt_, wps)
            wt_sb.append(wt_)

        # c0_col = M b1 + attn b4 (per block)
        c0_col = []
        for b in range(2):
            cps = ap.tile([P, 1], F32, name="cps", tag="alg")
            for k in range(2):
                nc.tensor.matmul(cps, mt_sb[k][:, b * P:(b + 1) * P],
                                 b1_col[k], start=(k == 0), stop=False)
            for k in range(2):
                nc.tensor.matmul(cps, attnT_sb[k][:, b * P:(b + 1) * P],
                                 b4_col[k], start=False, stop=(k == 1))
            ct = alg_sb.tile([P, 1], F32, name=f"c0_col{b}", tag=f"c0_col{b}")
            nc.vector.tensor_copy(ct, cps)
            c0_col.append(ct)

    # --- pass 2: out = W x + c0 1^T, fp16 out ---------------------------
    with tc.tile_pool(name="o_ps", bufs=4, space="PSUM") as ops, \
         tc.tile_pool(name="o_sb", bufs=3) as osb:
        nsub = PIECE // NT  # psum tiles per staging tile
        for i in range(NPIECE):
            for b in range(2):
                ot = osb.tile([P, PIECE], F16, name="ot", tag="ot")
                pst = [
                    ops.tile([P, NT], F32, name="pst", tag="pst")
                    for _ in range(nsub)
                ]
                for k in range(2):
                    for t in range(nsub):
                        nc.tensor.matmul(
                            pst[t],
                            wt_sb[k][:, b * P:(b + 1) * P],
                            xs[k][i][:, t * NT:(t + 1) * NT],
                            start=(k == 0),
                            stop=(k == 1),
                        )
                for t in range(nsub):
                    eng = (nc.scalar, nc.vector)[(2 * i + b) % 2]
                    if eng is nc.scalar:
                        eng.activation(
                            out=ot[:, t * NT:(t + 1) * NT], in_=pst[t],
                            func=mybir.ActivationFunctionType.Identity,
                            bias=c0_col[b], scale=1.0,
                        )
                    else:
                        eng.tensor_scalar_add(ot[:, t * NT:(t + 1) * NT],
                                              pst[t], c0_col[b])
                qs[(2 * i + b) % 2].dma_start(
                    out=out_d[b * P:(b + 1) * P, i * PIECE:(i + 1) * PIECE],
                    in_=ot,
                )


def build_program(enable_asserts=False):
    nc = bacc.Bacc(
        "TRN2",
        target_bir_lowering=False,
        debug=False,
        enable_asserts=enable_asserts,
        num_devices=8,
    )
    d_in = {
        "xht": nc.dram_tensor("xht", [NPIECE, P, CH_PP, C + 1],
                              F16, kind="ExternalInput").ap(),
        "wcat": nc.dram_tensor("wcat", [C, 5 * C], F32,
                               kind="ExternalInput").ap(),
        "brows": nc.dram_tensor("brows", [5, C], F32,
                                kind="ExternalInput").ap(),
        "bcols": nc.dram_tensor("bcols", [C, 4], F32,
                                kind="ExternalInput").ap(),
        "ident": nc.dram_tensor("ident", [P, P], F32,
                                kind="ExternalInput").ap(),
        "ident16": nc.dram_tensor("ident16", [P, P], F16,
                                  kind="ExternalInput").ap(),
    }
    d_out = {
        "out": nc.dram_tensor("out", [C, NPIX], F16,
                              kind="ExternalOutput").ap(),
    }
    with tile.TileContext(nc) as tc, ExitStack() as ctx:
        _emit(nc, tc, ctx, d_in, d_out)
    nc.compile()
    return nc


def make_in_maps(a, b, w1, b1, w2, b2, w3, b3, w4, b4):
    N = NPIX
    f = np.float32
    wcat = np.concatenate([w1.T, w1, w2.T, w3.T, w4],
                          axis=1).astype(f, copy=False)
    brows = np.stack([b1, N * b1, b2, b3, N * b3]).astype(f, copy=False)
    bcols = np.stack([b1, N * b1, b4, np.ones(C, f)], axis=1).astype(f)
    ident = np.eye(P, dtype=f)
    ident16 = np.eye(P, dtype=np.float16)
    B = a.shape[0]
    in_maps = []
    for i in range(B):
        x = np.concatenate([a[i].reshape(P, N), b[i].reshape(P, N)], axis=0)
        xh = x.astype(np.float16)
        xht = np.ascontiguousarray(
            xh.T.reshape(NPIECE, CH_PP, P, C).transpose(0, 2, 1, 3))
        ones = np.ones((NPIECE, P, CH_PP, 1), np.float16)
        xht = np.ascontiguousarray(np.concatenate([xht, ones], axis=3))
        in_maps.append({
            "xht": xht,
            "wcat": wcat,
            "brows": brows,
            "bcols": bcols,
            "ident": ident,
            "ident16": ident16,
        })
    return in_maps


_CACHE = {}


def kernel(a, b, w1, b1, w2, b2, w3, b3, w4, b4, _trace=False):
    a = np.asarray(a, dtype=np.float32)
    b = np.asarray(b, dtype=np.float32)
    args = [np.asarray(t, dtype=np.float32)
            for t in (w1, b1, w2, b2, w3, b3, w4, b4)]
    if "nc" not in _CACHE:
        _CACHE["nc"] = build_program()
    nc = _CACHE["nc"]
    in_maps = make_in_maps(a, b, *args)
    res = run_bass_kernel_spmd(nc, in_maps, core_ids=list(range(8)),
                               trace=_trace)
    B, Ch, H, W = a.shape
    out = np.stack([np.asarray(r["out"], dtype=np.float32).reshape(C, H, W)
                    for r in res.results])
    if _trace:
        _CACHE["last_results"] = res
    return out


# revision 12
# speedup vs baseline: 1.8841x; 1.2020x over previous
"""Trainium2 Bass kernel for nn_CLF_block (channel-attention block).

Reference computation (per batch item, with x = concat([a,b], ch) in [256, N],
N = H*W = 16384):
    z  = w1 x + b1 1^T
    q  = w2 z + b2 1^T ;  k = w3 z + b3 1^T ;  v = w4 z + b4 1^T
    qk = q k^T ; attn = softmax(qk, -1) ; out = attn v

Weight folding (host): A = w2 w1, B = w3 w1, Cw = w4 w1,
beta2 = w2 b1 + b2, beta3 = w3 b1 + b3, beta4 = w4 b1 + b4.  Then with
Gx = x x^T and sx = x 1 (one streaming pass over x):
    qk   = A Gx B^T + (A sx) beta3^T + beta2 (B sx)^T + N beta2 beta3^T
    attn = softmax(qk)
    W    = attn Cw ; c0 = attn beta4
    out  = W x + c0 1^T          (second streaming pass over x)

Data plan (per core; tolerance is 2e-2 so fp16 x suffices end-to-end,
numpy-verified rel err ~2.9e-3):
  - x streams once as transposed fp16 pieces xht [n, c] with a ones column
    (8.4 MiB, 8 KiB DMA lines); the Gram accumulates directly from them.
  - x also streams once in natural fp16 layout (8.4 MiB, 8 KiB lines) as the
    pass-2 moving operand (cheaper than on-chip PE transposes, which starve
    the PE sequencer).
  - Output is written as fp16 (8.4 MiB, 4 KiB lines) and upcast on host.

Sharding: data-parallel over batch, one batch item per NeuronCore (B=8).
"""

import sys

if "/opt/trn_rl_repo" not in sys.path:
    sys.path.insert(0, "/opt/trn_rl_repo")

from contextlib import ExitStack

import numpy as np

import concourse.bass as bass
import concourse.mybir as mybir
import concourse.tile as tile
from concourse import bacc
from concourse.bass_utils import run_bass_kernel_spmd

F32 = mybir.dt.float32
F32R = mybir.dt.float32r
F16 = mybir.dt.float16
P = 128          # partitions / channel block
C = 256          # channels
NPIX = 128 * 128  # spatial positions per batch item
NPIECE = 8        # streamed x^T pieces
PIECE = NPIX // NPIECE   # 2048 cols per piece
NCHUNK = NPIX // P       # 128 gram chunks
CH_PP = PIECE // P       # gram chunks per piece (16)
NT = 512          # matmul moving-operand width for pass 2
OG = 2048         # output staging width (4 KiB fp16 lines)


def _emit(nc, tc, ctx, d_in, d_out):
    """Emit the Tile program for one core (one batch item)."""
    wcat, ident = d_in["wcat"], d_in["ident"]
    xht_d, xnat_d = d_in["xht"], d_in["xnat"]
    brows, bcols = d_in["brows"], d_in["bcols"]
    out_d = d_out["out"]

    const = ctx.enter_context(tc.tile_pool(name="const", bufs=1))
    xpool = ctx.enter_context(tc.tile_pool(name="xpool", bufs=1))

    qs = [nc.sync, nc.scalar]

    # --- constants -------------------------------------------------------
    # wcat columns: [A^T | B^T | Cw] as two 128-row blocks, f32r for the
    # fast-path PE matmuls (1 cyc/row at >=256 moving columns).
    w_sb = []
    for k in range(2):
        wt = const.tile([P, 3 * C], F32R, name=f"w_sb{k}", tag=f"w_sb{k}")
        nc.sync.dma_start(out=wt, in_=wcat[k * P:(k + 1) * P, :])
        w_sb.append(wt)
    at_ = [w_sb[k][:, 0 * C:1 * C] for k in range(2)]   # A^T  [c', o]
    bt_ = [w_sb[k][:, 1 * C:2 * C] for k in range(2)]   # B^T  [c', o]
    cw_ = [w_sb[k][:, 2 * C:3 * C] for k in range(2)]   # Cw   [d, c']

    rows = []
    for r in range(3):
        rt = const.tile([1, C], F32, name=f"brow{r}", tag=f"brow{r}")
        nc.sync.dma_start(out=rt, in_=brows[r:r + 1, :])
        rows.append(rt)
    b2_row, b3_row, nb3_row = rows

    b4_col = []
    for k in range(2):
        bt = const.tile([P, 1], F32, name=f"bcol{k}", tag=f"bcol{k}")
        nc.sync.dma_start(out=bt, in_=bcols[k * P:(k + 1) * P, :])
        b4_col.append(bt)

    ident_sb = const.tile([P, P], F32, name="ident_sb", tag="ident_sb")
    nc.sync.dma_start(out=ident_sb, in_=ident[:, :])

    # --- natural-layout fp16 x (pass-2 moving operand), 4 big tiles/block
    spj = NPIX // 4         # columns per resident x tile
    xs = [[], []]
    for k in range(2):
        for j in range(4):
            xt = xpool.tile([P, spj], F16, name=f"x{k}_{j}", tag=f"x{k}_{j}")
            qs[(k * 4 + j) % 2].dma_start(
                out=xt, in_=xnat_d[k, :, j * spj:(j + 1) * spj])
            xs[k].append(xt)

    # --- pass 1: stream x^T pieces; Gram [both 128-row blocks, full width]
    gx_sb = [
        const.tile([P, C + 1], F32R, name=f"gx_sb{b}", tag=f"gx_sb{b}")
        for b in range(2)
    ]
    with tc.tile_pool(name="gx_ps", bufs=1, space="PSUM") as gxp, \
         tc.tile_pool(name="xt_sb", bufs=3) as xtp:
        shh = [
            gxp.tile([P, C + 1], F32, name=f"shh{b}", tag=f"shh{b}")
            for b in range(2)
        ]
        for i in range(NPIECE):
            xht_p = xtp.tile([P, CH_PP, C + 1], F16, name="xht_p", tag="xht_p")
            qs[i % 2].dma_start(out=xht_p, in_=xht_d[i])
            for g in range(CH_PP):
                ch = i * CH_PP + g
                chunk = xht_p[:, g, :]
                for b in range(2):
                    nc.tensor.matmul(shh[b], chunk[:, b * P:(b + 1) * P],
                                     chunk,
                                     start=(ch == 0), stop=(ch == NCHUNK - 1))
        nc.vector.tensor_copy(gx_sb[0], shh[0])
        nc.vector.tensor_copy(gx_sb[1], shh[1])

    # Split the (large) diagonal out of Gx so the f32r algebra matmuls see
    # well-scaled operands; the diagonal term is re-applied exactly via
    # per-partition multiplies.
    gxd = []
    for b in range(2):
        bs = slice(b * P, (b + 1) * P)
        dm = const.tile([P, P], F32, name=f"gxdm{b}", tag=f"gxdm{b}")
        nc.vector.tensor_mul(dm, gx_sb[b].bitcast(F32)[:, bs], ident_sb)
        dcol = const.tile([P, 1], F32, name=f"gxd{b}", tag=f"gxd{b}")
        nc.vector.reduce_sum(out=dcol, in_=dm, axis=mybir.AxisListType.X)
        nc.vector.tensor_sub(gx_sb[b][:, bs],
                             gx_sb[b].bitcast(F32)[:, bs], dm)
        gxd.append(dcol)

    # --- tiny 256x256 algebra -------------------------------------------
    alg_sb = const

    with tc.tile_pool(name="alg_ps", bufs=3, space="PSUM") as ap:
        # asx = (A sx)^T, bsx = (B sx)^T (sx sits in gx col 256)
        asx_row = alg_sb.tile([1, C], F32, name="asx_row", tag="asx_row")
        bsx_row = alg_sb.tile([1, C], F32, name="bsx_row", tag="bsx_row")
        for dst, wt in ((asx_row, at_), (bsx_row, bt_)):
            vps = ap.tile([1, C], F32, name="vps", tag="algsmall", bufs=2)
            for k in range(2):
                nc.tensor.matmul(vps, gx_sb[k][:, C:C + 1], wt[k],
                                 start=(k == 0), stop=(k == 1))
            nc.vector.tensor_copy(dst, vps)

        # U2 = (A Gx)^T = Gx~ A^T + D A^T
        u2_sb = []
        for b in range(2):
            u2ps = ap.tile([P, C], F32, name="u2ps", tag="alg")
            for k in range(2):
                nc.tensor.matmul(u2ps, gx_sb[k][:, b * P:(b + 1) * P], at_[k],
                                 start=(k == 0), stop=(k == 1))
            u2d = alg_sb.tile([P, C], F32, name=f"u2_d{b}", tag=f"u2_d{b}")
            nc.vector.tensor_scalar_mul(u2d, at_[b].bitcast(F32), gxd[b])
            u2t = alg_sb.tile([P, C], F32R, name=f"u2_sb{b}", tag=f"u2_sb{b}")
            nc.vector.tensor_add(u2t, u2ps, u2d)
            u2_sb.append(u2t)

        # qk = U2^T B^T + rank-1 terms ; then softmax rows
        attn_sb = []
        for b in range(2):
            qkps = ap.tile([P, C], F32, name="qkps", tag="alg")
            for k in range(2):
                nc.tensor.matmul(qkps, u2_sb[k][:, b * P:(b + 1) * P], bt_[k],
                                 start=(k == 0), stop=False)
            nc.tensor.matmul(qkps, asx_row[:, b * P:(b + 1) * P], b3_row,
                             start=False, stop=False)
            nc.tensor.matmul(qkps, b2_row[:, b * P:(b + 1) * P], bsx_row,
                             start=False, stop=False)
            nc.tensor.matmul(qkps, b2_row[:, b * P:(b + 1) * P],
                             nb3_row, start=False, stop=True)

            negmax = alg_sb.tile([P, 1], F32, name=f"negmax{b}", tag=f"nm{b}")
            nc.vector.tensor_reduce(
                out=negmax, in_=qkps, op=mybir.AluOpType.max,
                axis=mybir.AxisListType.X, negate=True,
            )
            expq = alg_sb.tile([P, C], F32, name=f"expq{b}", tag=f"expq{b}")
            nc.scalar.activation(
                out=expq, in_=qkps, func=mybir.ActivationFunctionType.Exp,
                bias=negmax, scale=1.0,
            )
            denom = alg_sb.tile([P, 1], F32, name=f"denom{b}", tag=f"dn{b}")
            nc.vector.reduce_sum(out=denom, in_=expq,
                                 axis=mybir.AxisListType.X)
            rden = alg_sb.tile([P, 1], F32, name=f"rden{b}", tag=f"rd{b}")
            nc.vector.reciprocal(rden, denom)
            at = alg_sb.tile([P, C], F32, name=f"attn{b}", tag=f"attn{b}")
            nc.vector.tensor_scalar_mul(at, expq, rden)
            attn_sb.append(at)

        # attn^T (4 PE transposes)
        attnT_sb = [
            alg_sb.tile([P, C], F32R, name=f"attnT{j}", tag=f"attnT{j}")
            for j in range(2)
        ]
        for b in range(2):
            for j in range(2):
                tps = ap.tile([P, P], F32, name="tps", tag="algtp", bufs=2)
                nc.tensor.transpose(tps, attn_sb[b][:, j * P:(j + 1) * P],
                                    ident_sb)
                nc.vector.tensor_copy(
                    attnT_sb[j][:, b * P:(b + 1) * P], tps)

        # W^T = Cw-as-lhsT @ attn^T  (stored fp16 for the pass-2 matmuls)
        wt_sb = []
        for b in range(2):
            wps = ap.tile([P, C], F32, name="wps", tag="alg")
            for k in range(2):
                nc.tensor.matmul(wps, cw_[k][:, b * P:(b + 1) * P],
                                 attnT_sb[k], start=(k == 0), stop=(k == 1))
            wt_ = alg_sb.tile([P, C], F16, name=f"wt_sb{b}", tag=f"wt_sb{b}")
            nc.vector.tensor_copy(wt_, wps)
            wt_sb.append(wt_)

        # c0_col = attn beta4 (per block)
        c0_col = []
        for b in range(2):
            cps = ap.tile([P, 1], F32, name="cps", tag="algsmall", bufs=2)
            for k in range(2):
                nc.tensor.matmul(cps,
                                 attnT_sb[k][:, b * P:(b + 1) * P].bitcast(F32),
                                 b4_col[k], start=(k == 0), stop=(k == 1))
            ct = alg_sb.tile([P, 1], F32, name=f"c0_col{b}", tag=f"c0_col{b}")
            nc.scalar.copy(ct, cps)
            c0_col.append(ct)

    # --- pass 2: out = W x + c0 1^T, fp16 out, 4 KiB DMA lines -----------
    with tc.tile_pool(name="o_ps", bufs=8, space="PSUM") as ops, \
         tc.tile_pool(name="o_sb", bufs=3) as osb:
        ngrp = NPIX // OG       # 8 output groups of OG columns
        nsub = OG // NT         # 4 psum tiles per staging tile
        for i in range(ngrp):
            for b in range(2):
                ot = osb.tile([P, OG], F16, name="ot", tag="ot")
                pst = [
                    ops.tile([P, NT], F32, name="pst", tag="pst")
                    for _ in range(nsub)
                ]
                for k in range(2):
                    for t in range(nsub):
                        col = i * OG + t * NT
                        nc.tensor.matmul(
                            pst[t],
                            wt_sb[k][:, b * P:(b + 1) * P],
                            xs[k][col // spj][:, col % spj:col % spj + NT],
                            start=(k == 0),
                            stop=(k == 1),
                        )
                for t in range(nsub):
                    eng = (nc.scalar, nc.vector)[(i + b + t) % 2]
                    if eng is nc.scalar:
                        eng.activation(
                            out=ot[:, t * NT:(t + 1) * NT], in_=pst[t],
                            func=mybir.ActivationFunctionType.Identity,
                            bias=c0_col[b], scale=1.0,
                        )
                    else:
                        eng.tensor_scalar_add(ot[:, t * NT:(t + 1) * NT],
                                              pst[t], c0_col[b])
                qs[(2 * i + b) % 2].dma_start(
                    out=out_d[b * P:(b + 1) * P, i * OG:(i + 1) * OG],
                    in_=ot,
                )


def build_program(enable_asserts=False):
    nc = bacc.Bacc(
        "TRN2",
        target_bir_lowering=False,
        debug=False,
        enable_asserts=enable_asserts,
        num_devices=8,
    )
    d_in = {
        "xht": nc.dram_tensor("xht", [NPIECE, P, CH_PP, C + 1],
                              F16, kind="ExternalInput").ap(),
        "xnat": nc.dram_tensor("xnat", [2, P, NPIX], F16,
                               kind="ExternalInput").ap(),
        "wcat": nc.dram_tensor("wcat", [C, 3 * C], F32R,
                               kind="ExternalInput").ap(),
        "brows": nc.dram_tensor("brows", [3, C], F32,
                                kind="ExternalInput").ap(),
        "bcols": nc.dram_tensor("bcols", [C, 1], F32,
                                kind="ExternalInput").ap(),
        "ident": nc.dram_tensor("ident", [P, P], F32,
                                kind="ExternalInput").ap(),
    }
    d_out = {
        "out": nc.dram_tensor("out", [C, NPIX], F16,
                              kind="ExternalOutput").ap(),
    }
    with tile.TileContext(nc) as tc, ExitStack() as ctx:
        _emit(nc, tc, ctx, d_in, d_out)
    nc.compile()
    return nc


def _round_f32r(x):
    """Round fp32 to the FP32R-representable set (hi-bf16 + lo-bf16)."""
    import ml_dtypes

    x = np.asarray(x, np.float32)
    hi = x.astype(ml_dtypes.bfloat16).astype(np.float32)
    lo = (x - hi).astype(ml_dtypes.bfloat16).astype(np.float32)
    return hi + lo


def make_in_maps(a, b, w1, b1, w2, b2, w3, b3, w4, b4):
    N = NPIX
    f = np.float32
    A = (w2.astype(np.float64) @ w1.astype(np.float64)).astype(f)
    B_ = (w3.astype(np.float64) @ w1.astype(np.float64)).astype(f)
    Cw = (w4.astype(np.float64) @ w1.astype(np.float64)).astype(f)
    be2 = (w2 @ b1 + b2).astype(f)
    be3 = (w3 @ b1 + b3).astype(f)
    be4 = (w4 @ b1 + b4).astype(f)
    wcat = _round_f32r(np.concatenate([A.T, B_.T, Cw], axis=1))
    brows = np.stack([be2, be3, N * be3]).astype(f, copy=False)
    bcols = np.ascontiguousarray(be4[:, None].astype(f))
    ident = np.eye(P, dtype=f)
    in_maps = []
    for i in range(a.shape[0]):
        x = np.concatenate([a[i].reshape(P, N), b[i].reshape(P, N)], axis=0)
        xh = x.astype(np.float16)
        xht = np.ascontiguousarray(
            xh.T.reshape(NPIECE, CH_PP, P, C).transpose(0, 2, 1, 3))
        ones = np.ones((NPIECE, P, CH_PP, 1), np.float16)
        xht = np.ascontiguousarray(np.concatenate([xht, ones], axis=3))
        in_maps.append({
            "xht": xht,
            "xnat": np.ascontiguousarray(xh.reshape(2, P, N)),
            "wcat": wcat,
            "brows": brows,
            "bcols": bcols,
            "ident": ident,
        })
    return in_maps


_CACHE = {}


def kernel(a, b, w1, b1, w2, b2, w3, b3, w4, b4, _trace=False):
    a = np.asarray(a, dtype=np.float32)
    b = np.asarray(b, dtype=np.float32)
    args = [np.asarray(t, dtype=np.float32)
            for t in (w1, b1, w2, b2, w3, b3, w4, b4)]
    if "nc" not in _CACHE:
        _CACHE["nc"] = build_program()
    nc = _CACHE["nc"]
    in_maps = make_in_maps(a, b, *args)
    res = run_bass_kernel_spmd(nc, in_maps, core_ids=list(range(8)),
                               trace=_trace)
    B, Ch, H, W = a.shape
    out = np.stack([np.asarray(r["out"], dtype=np.float32).reshape(C, H, W)
                    for r in res.results])
    if _trace:
        _CACHE["last_results"] = res
    return out
